# revision 2
# baseline (speedup 1.0000x reference)
"""EMAVectorQuantizer forward on 8 Trainium2 NeuronCores (Bass/Tile).

Reference computation (see problem):
    d[n,k] = ||z_n||^2 + ||w_k||^2 - 2 z_n.w_k          n<65536, k<1024, D=256
    idx[n] = argmin_k d[n,k]   (first occurrence)
    z_q    = w[idx]
    loss   = 0.25 * mean((z_q - z)^2)
    z_q_st = z + (z_q - z)
    returns (loss, z_q_st, idx)

Strategy (data parallel over tokens, 8 cores x 8192 tokens):
    argmin_k d = argmax_k q where q[n,k] = 2 z_n.w_k - ||w_k||^2
    (the ||z||^2 term is constant per row).
  Per 128-token tile on each core:
    - PE: q = [zT | ones]^T @ [2 w^T ; -||w||^2]  (K=256+1 contraction,
      bias folded in as an extra contraction row) -> PSUM [128,1024] f32
    - DVE tensor_tensor_scan (op0=max): pm = prefix-max of q -> SBUF.
      pm[:, -1] is the row max (= -min d + ||z||^2 term dropped).
    - DVE tensor_scalar accum: idx = sum_k(pm[:,k] < pm[:,-1]) — exact
      first-occurrence argmax (prefix-max crosses the max exactly at the
      first occurrence; all f32 compares exact).
    - GPSIMD indirect DMA: z_q rows gathered from weight in HBM by idx.
  loss from row maxes: sum d_min = sum z^2 - sum_n max_k q  (host f64 final
  reduction of per-core partials).
"""

import numpy as np

N_TOK = 65536
K_CODES = 1024
D_DIM = 256
N_CORES = 8
TOK_PER_CORE = N_TOK // N_CORES          # 8192
TILES = TOK_PER_CORE // 128              # 64
BETA = 0.25

_cache = {}


def _install_birpatch():
    """Split multi-wait instructions in the BIR: this walrus build allows only
    one sem-wait per instruction, but Tile's kernel-tail drain carries one
    wait per semaphore lane. Extra waits move to single-wait Drain carriers."""
    if _cache.get("birpatch"):
        return
    _cache["birpatch"] = True
    import json as _json

    import concourse.bass_utils as _bu
    import concourse.bass2jax as _b2j

    def _split_multiwait(bir_bytes):
        m = _json.loads(bir_bytes)
        changed = False
        for fn in m.get("functions", []):
            for bb in fn.get("blocks", []):
                out = []
                for inst in bb.get("instructions", []):
                    si = inst.get("sync_info") or {}
                    waits = si.get("on_wait") or []
                    if len(waits) <= 1:
                        out.append(inst)
                        continue
                    changed = True
                    for i, w in enumerate(waits[:-1]):
                        out.append(
                            {
                                "name": f"{inst['name']}-w{i}",
                                "opcode": "Drain",
                                "engine": inst["engine"],
                                "ins": [],
                                "outs": [],
                                "debug": inst.get("debug", 0),
                                "is_reset_sema": False,
                                "sync_info": {"on_update": [], "on_wait": [w]},
                            }
                        )
                    si["on_wait"] = [waits[-1]]
                    inst["sync_info"] = si
                    out.append(inst)
                bb["instructions"] = out
        return _json.dumps(m).encode() if changed else bir_bytes

    orig = _bu.compile_bir_kernel

    def patched(bir_json, tmpdir, neff_name="file.neff"):
        if isinstance(bir_json, str):
            bir_json = bir_json.encode()
        return orig(_split_multiwait(bir_json), tmpdir, neff_name)

    _bu.compile_bir_kernel = patched
    _b2j.compile_bir_kernel = patched


def _build_bass():
    _install_birpatch()
    import concourse.bass as bass
    import concourse.tile as tile
    import concourse.mybir as mybir

    f32 = mybir.dt.float32
    nc = bass.Bass()

    zT = nc.dram_tensor("zT", [D_DIM, TOK_PER_CORE], f32, kind="ExternalInput")
    wT2 = nc.dram_tensor("wT2", [D_DIM, K_CODES], f32, kind="ExternalInput")
    negwsq = nc.dram_tensor("negwsq", [1, K_CODES], f32, kind="ExternalInput")
    wsrc = nc.dram_tensor("wsrc", [K_CODES, D_DIM], f32, kind="ExternalInput")

    zq = nc.dram_tensor("zq", [TOK_PER_CORE, D_DIM], f32, kind="ExternalOutput")
    idxo = nc.dram_tensor("idxo", [128, TILES], mybir.dt.int32, kind="ExternalOutput")
    vmaxo = nc.dram_tensor("vmaxo", [128, TILES], f32, kind="ExternalOutput")

    with tile.TileContext(nc) as tc:
        with (
            tc.tile_pool(name="const", bufs=1) as cpool,
            tc.tile_pool(name="zin", bufs=6) as zpool,
            tc.tile_pool(name="psum", bufs=3, space="PSUM") as qpool,
            tc.tile_pool(name="pmbuf", bufs=3) as pmpool,
            tc.tile_pool(name="junk", bufs=2) as jpool,
            tc.tile_pool(name="small", bufs=1) as spool,
            tc.tile_pool(name="gat", bufs=4) as gpool,
        ):
            wa = cpool.tile([128, K_CODES], f32, tag="wa")
            nc.sync.dma_start(wa[:], wT2[0:128, :])
            wb = cpool.tile([128, K_CODES], f32, tag="wb")
            nc.sync.dma_start(wb[:], wT2[128:256, :])
            nw = cpool.tile([1, K_CODES], f32, tag="nw")
            nc.sync.dma_start(nw[:], negwsq[:])
            ones = cpool.tile([1, 128], f32, tag="ones")
            nc.vector.memset(ones[:], 1.0)
            zeros = cpool.tile([128, K_CODES], f32, tag="zeros")
            nc.vector.memset(zeros[:], 0.0)

            idxf = spool.tile([128, TILES], f32, tag="idxf")
            idxi = spool.tile([128, TILES], mybir.dt.int32, tag="idxi")
            vmaxb = spool.tile([128, TILES], f32, tag="vmaxb")

            for tt in range(TILES):
                zt0 = zpool.tile([128, 128], f32, tag="zt0")
                nc.sync.dma_start(zt0[:], zT[0:128, tt * 128 : (tt + 1) * 128])
                zt1 = zpool.tile([128, 128], f32, tag="zt1")
                nc.sync.dma_start(zt1[:], zT[128:256, tt * 128 : (tt + 1) * 128])

                q = qpool.tile([128, K_CODES], f32, tag="q")
                nc.tensor.matmul(q[:, 0:512], zt0[:], wa[:, 0:512], start=True, stop=False)
                nc.tensor.matmul(q[:, 512:1024], zt0[:], wa[:, 512:1024], start=True, stop=False)
                nc.tensor.matmul(q[:, 0:512], zt1[:], wb[:, 0:512], start=False, stop=False)
                nc.tensor.matmul(q[:, 512:1024], zt1[:], wb[:, 512:1024], start=False, stop=False)
                nc.tensor.matmul(q[:, 0:512], ones[0:1, :], nw[0:1, 0:512], start=False, stop=True)
                nc.tensor.matmul(q[:, 512:1024], ones[0:1, :], nw[0:1, 512:1024], start=False, stop=True)

                pm = pmpool.tile([128, K_CODES], f32, tag="pm")
                nc.vector.tensor_tensor_scan(
                    out=pm[:],
                    data0=q[:],
                    data1=zeros[:],
                    initial=-3.0e38,
                    op0=mybir.AluOpType.max,
                    op1=mybir.AluOpType.bypass,
                )
                junk = jpool.tile([128, K_CODES], f32, tag="junk")
                nc.vector.tensor_scalar(
                    out=junk[:],
                    in0=pm[:],
                    scalar1=pm[:, K_CODES - 1 : K_CODES],
                    scalar2=0.0,
                    op0=mybir.AluOpType.is_lt,
                    op1=mybir.AluOpType.add,
                    accum_out=idxf[:, tt : tt + 1],
                )
                # row max for the loss
                nc.scalar.copy(vmaxb[:, tt : tt + 1], pm[:, K_CODES - 1 : K_CODES])

            # f32 -> int32 (values in [0, 1023], exact)
            nc.vector.tensor_copy(idxi[:], idxf[:])

            import concourse.bass as _b

            for tt in range(TILES):
                g = gpool.tile([128, D_DIM], f32, tag="g")
                nc.gpsimd.indirect_dma_start(
                    out=g[:],
                    out_offset=None,
                    in_=wsrc[:],
                    in_offset=_b.IndirectOffsetOnAxis(ap=idxi[:, tt : tt + 1], axis=0),
                )
                nc.sync.dma_start(zq[tt * 128 : (tt + 1) * 128, :], g[:])

            nc.sync.dma_start(idxo[:], idxi[:])
            nc.sync.dma_start(vmaxo[:], vmaxb[:])

    return nc


def kernel(z: np.ndarray, weight: np.ndarray):
    z = np.ascontiguousarray(z, dtype=np.float32)
    weight = np.ascontiguousarray(weight, dtype=np.float32)
    assert z.shape == (N_TOK, D_DIM) and weight.shape == (K_CODES, D_DIM)

    if "nc" not in _cache:
        _cache["nc"] = _build_bass()
    nc = _cache["nc"]

    from concourse.bass_utils import run_bass_kernel_spmd

    zT_full = np.ascontiguousarray(z.T)                     # [256, 65536]
    wT2 = np.ascontiguousarray((2.0 * weight).T)            # [256, 1024]
    negwsq = np.ascontiguousarray(
        -(weight.astype(np.float64) ** 2).sum(axis=1).astype(np.float32)[None, :]
    )
    in_maps = []
    for c in range(N_CORES):
        in_maps.append(
            {
                "zT": np.ascontiguousarray(
                    zT_full[:, c * TOK_PER_CORE : (c + 1) * TOK_PER_CORE]
                ),
                "wT2": wT2,
                "negwsq": negwsq,
                "wsrc": weight,
            }
        )

    res = run_bass_kernel_spmd(nc, in_maps, core_ids=list(range(N_CORES)))
    return _assemble(z, weight, [res.results[c] for c in range(N_CORES)])


def _assemble(z, weight, results):
    z_q = np.concatenate([r["zq"] for r in results], axis=0)
    idx = np.concatenate(
        [r["idxo"].T.reshape(-1) for r in results], axis=0
    ).astype(np.int32)
    # loss = BETA * mean(d_min); d_min[n] = ||z_n||^2 - max_k q[n,k]
    sum_vmax = sum(float(r["vmaxo"].sum(dtype=np.float64)) for r in results)
    sum_zsq = float((z.astype(np.float64) ** 2).sum())
    loss = np.float32(BETA * (sum_zsq - sum_vmax) / (N_TOK * D_DIM))
    # straight-through estimator, matching reference arithmetic exactly
    z_q_st = z + (z_q - z)
    return (loss, z_q_st, idx)


# revision 5
# speedup vs baseline: 1.3114x; 1.3114x over previous
"""EMAVectorQuantizer forward on 8 Trainium2 NeuronCores (Bass/Tile).

Reference computation:
    d[n,k] = ||z_n||^2 + ||w_k||^2 - 2 z_n.w_k          n<65536, k<1024, D=256
    idx[n] = argmin_k d[n,k]   (first occurrence)
    z_q    = w[idx];  loss = 0.25*mean((z_q - z)^2);  z_q_st = z + (z_q - z)

Strategy (data parallel over tokens, 8 cores x 8192 tokens):
  argmin_k d = argmax_k q,  q[n,k] = 2 z_n.w_k - ||w_k||^2  (||z||^2 drops).

  Matmul precision: the PE's fast fp32 mode (float32r) rounds both operands
  to ~10 mantissa bits (TF32-like) but multiplies those exactly.  We split
  each operand at 10 mantissa bits (x = xh + xl, xh = trunc10(x)) and
  compute q = zh.wh + zh.wl + zl.wh (+ split bias rows) — every partial
  product is exact to f32, accumulation is f32 in PSUM, the dropped zl.wl
  term is ~1e-5 absolute.  f32-grade accuracy at 1 cycle/row instead of
  native fp32's 4 cycles/row.

  Per 128-token tile:
    - PE: 16 matmuls (fp32r) -> q in PSUM [128,1024] f32
    - DVE tensor_tensor_scan (op0=max): pm = prefix-max of q -> SBUF;
      pm[:,-1] is the row max.
    - ACT: idx = accum(Sign(tmax - pm)) — counts elements before the first
      occurrence of the max (exact f32 compares, exact tie semantics);
      also copies pm[:,-1] (row max, used for the loss) to an output buffer.
    - GPSIMD indirect DMA: z_q rows gathered from weight in HBM by idx.
  loss: sum d_min = sum z^2 - sum_n max_k q  (host f64 final reduction).
"""

import numpy as np

N_TOK = 65536
K_CODES = 1024
D_DIM = 256
N_CORES = 8
TOK_PER_CORE = N_TOK // N_CORES          # 8192
TILES = TOK_PER_CORE // 128              # 64
BETA = 0.25

_cache = {}


def _install_birpatch():
    """Split multi-wait instructions in the BIR: this walrus build allows only
    one sem-wait per instruction, but Tile's kernel-tail drain carries one
    wait per semaphore lane. Extra waits move to single-wait Drain carriers."""
    if _cache.get("birpatch"):
        return
    _cache["birpatch"] = True
    import json as _json

    import concourse.bass_utils as _bu
    import concourse.bass2jax as _b2j

    def _split_multiwait(bir_bytes):
        m = _json.loads(bir_bytes)
        changed = False
        for fn in m.get("functions", []):
            for bb in fn.get("blocks", []):
                out = []
                for inst in bb.get("instructions", []):
                    si = inst.get("sync_info") or {}
                    waits = si.get("on_wait") or []
                    if len(waits) <= 1:
                        out.append(inst)
                        continue
                    changed = True
                    for i, w in enumerate(waits[:-1]):
                        out.append(
                            {
                                "name": f"{inst['name']}-w{i}",
                                "opcode": "Drain",
                                "engine": inst["engine"],
                                "ins": [],
                                "outs": [],
                                "debug": inst.get("debug", 0),
                                "is_reset_sema": False,
                                "sync_info": {"on_update": [], "on_wait": [w]},
                            }
                        )
                    si["on_wait"] = [waits[-1]]
                    inst["sync_info"] = si
                    out.append(inst)
                bb["instructions"] = out
        return _json.dumps(m).encode() if changed else bir_bytes

    orig = _bu.compile_bir_kernel

    def patched(bir_json, tmpdir, neff_name="file.neff"):
        if isinstance(bir_json, str):
            bir_json = bir_json.encode()
        return orig(_split_multiwait(bir_json), tmpdir, neff_name)

    _bu.compile_bir_kernel = patched
    _b2j.compile_bir_kernel = patched


def _build_bass():
    _install_birpatch()
    import concourse.bass as bass
    import concourse.tile as tile
    import concourse.mybir as mybir

    f32 = mybir.dt.float32
    r32 = mybir.dt.float32r
    nc = bass.Bass()

    zhT = nc.dram_tensor("zhT", [D_DIM, TOK_PER_CORE], r32, kind="ExternalInput")
    zlT = nc.dram_tensor("zlT", [D_DIM, TOK_PER_CORE], r32, kind="ExternalInput")
    whT = nc.dram_tensor("whT", [D_DIM, K_CODES], r32, kind="ExternalInput")
    wlT = nc.dram_tensor("wlT", [D_DIM, K_CODES], r32, kind="ExternalInput")
    nwh = nc.dram_tensor("nwh", [1, K_CODES], r32, kind="ExternalInput")
    nwl = nc.dram_tensor("nwl", [1, K_CODES], r32, kind="ExternalInput")
    wsrc = nc.dram_tensor("wsrc", [K_CODES, D_DIM], f32, kind="ExternalInput")

    zq = nc.dram_tensor("zq", [TOK_PER_CORE, D_DIM], f32, kind="ExternalOutput")
    idxo = nc.dram_tensor("idxo", [128, TILES], mybir.dt.int32, kind="ExternalOutput")
    vmaxo = nc.dram_tensor("vmaxo", [128, TILES], f32, kind="ExternalOutput")

    with tile.TileContext(nc) as tc:
        with (
            tc.tile_pool(name="const", bufs=1) as cpool,
            tc.tile_pool(name="zin", bufs=8) as zpool,
            tc.tile_pool(name="psum", bufs=3, space="PSUM") as qpool,
            tc.tile_pool(name="pmbuf", bufs=3) as pmpool,
            tc.tile_pool(name="junk", bufs=2) as jpool,
            tc.tile_pool(name="small", bufs=1) as spool,
            tc.tile_pool(name="gat", bufs=4) as gpool,
        ):
            wha = cpool.tile([128, K_CODES], r32, tag="wha")
            nc.sync.dma_start(wha[:], whT[0:128, :])
            whb = cpool.tile([128, K_CODES], r32, tag="whb")
            nc.sync.dma_start(whb[:], whT[128:256, :])
            wla = cpool.tile([128, K_CODES], r32, tag="wla")
            nc.sync.dma_start(wla[:], wlT[0:128, :])
            wlb = cpool.tile([128, K_CODES], r32, tag="wlb")
            nc.sync.dma_start(wlb[:], wlT[128:256, :])
            nh = cpool.tile([1, K_CODES], r32, tag="nh")
            nc.sync.dma_start(nh[:], nwh[:])
            nl = cpool.tile([1, K_CODES], r32, tag="nl")
            nc.sync.dma_start(nl[:], nwl[:])
            ones_t = cpool.tile([1, 128], f32, tag="ones")
            nc.vector.memset(ones_t[:], 1.0)
            ones = ones_t[:].bitcast(r32)
            zeros = cpool.tile([128, K_CODES], f32, tag="zeros")
            nc.vector.memset(zeros[:], 0.0)

            idxf = spool.tile([128, TILES], f32, tag="idxf")
            idxi = spool.tile([128, TILES], mybir.dt.int32, tag="idxi")
            vmaxb = spool.tile([128, TILES], f32, tag="vmaxb")

            for tt in range(TILES):
                sl = slice(tt * 128, (tt + 1) * 128)
                zh0 = zpool.tile([128, 128], r32, tag="zh0")
                nc.sync.dma_start(zh0[:], zhT[0:128, sl])
                zh1 = zpool.tile([128, 128], r32, tag="zh1")
                nc.sync.dma_start(zh1[:], zhT[128:256, sl])
                zl0 = zpool.tile([128, 128], r32, tag="zl0")
                nc.sync.dma_start(zl0[:], zlT[0:128, sl])
                zl1 = zpool.tile([128, 128], r32, tag="zl1")
                nc.sync.dma_start(zl1[:], zlT[128:256, sl])

                q = qpool.tile([128, K_CODES], f32, tag="q")
                L, R = slice(0, 512), slice(512, 1024)
                mm = nc.tensor.matmul
                # grouped by stationary operand (5 weight loads per tile)
                mm(q[:, L], zh0[:], wha[:, L], start=True, stop=False)
                mm(q[:, R], zh0[:], wha[:, R], start=True, stop=False)
                mm(q[:, L], zh0[:], wla[:, L], start=False, stop=False)
                mm(q[:, R], zh0[:], wla[:, R], start=False, stop=False)
                mm(q[:, L], zh1[:], whb[:, L], start=False, stop=False)
                mm(q[:, R], zh1[:], whb[:, R], start=False, stop=False)
                mm(q[:, L], zh1[:], wlb[:, L], start=False, stop=False)
                mm(q[:, R], zh1[:], wlb[:, R], start=False, stop=False)
                mm(q[:, L], zl0[:], wha[:, L], start=False, stop=False)
                mm(q[:, R], zl0[:], wha[:, R], start=False, stop=False)
                mm(q[:, L], zl1[:], whb[:, L], start=False, stop=False)
                mm(q[:, R], zl1[:], whb[:, R], start=False, stop=False)
                mm(q[:, L], ones[0:1, :], nh[0:1, L], start=False, stop=False)
                mm(q[:, R], ones[0:1, :], nh[0:1, R], start=False, stop=False)
                mm(q[:, L], ones[0:1, :], nl[0:1, L], start=False, stop=True)
                mm(q[:, R], ones[0:1, :], nl[0:1, R], start=False, stop=True)

                pm = pmpool.tile([128, K_CODES], f32, tag="pm")
                nc.vector.tensor_tensor_scan(
                    out=pm[:],
                    data0=q[:],
                    data1=zeros[:],
                    initial=-3.0e38,
                    op0=mybir.AluOpType.max,
                    op1=mybir.AluOpType.bypass,
                )
                # idx = sum_k Sign(tmax - pm[k]) on the scalar engine
                junk = jpool.tile([128, K_CODES], f32, tag="junk")
                nc.scalar.activation(
                    junk[:],
                    pm[:],
                    mybir.ActivationFunctionType.Sign,
                    bias=pm[:, K_CODES - 1 : K_CODES],
                    scale=-1.0,
                    accum_out=idxf[:, tt : tt + 1],
                )
                # row max for the loss
                nc.scalar.copy(vmaxb[:, tt : tt + 1], pm[:, K_CODES - 1 : K_CODES])

            # f32 -> int32 (values in [0, 1023], exact)
            nc.vector.tensor_copy(idxi[:], idxf[:])

            for tt in range(TILES):
                g = gpool.tile([128, D_DIM], f32, tag="g")
                nc.gpsimd.indirect_dma_start(
                    out=g[:],
                    out_offset=None,
                    in_=wsrc[:],
                    in_offset=bass.IndirectOffsetOnAxis(
                        ap=idxi[:, tt : tt + 1], axis=0
                    ),
                )
                nc.sync.dma_start(zq[tt * 128 : (tt + 1) * 128, :], g[:])

            nc.sync.dma_start(idxo[:], idxi[:])
            nc.sync.dma_start(vmaxo[:], vmaxb[:])

    return nc


def _trunc10(x):
    return (x.view(np.int32) & np.int32(~((1 << 13) - 1))).view(np.float32)


def _prep_inputs(z, weight):
    zT = np.ascontiguousarray(z.T)                         # [256, 65536]
    zhT_full = _trunc10(zT)
    zlT_full = zT - zhT_full
    w2T = np.ascontiguousarray((2.0 * weight).T)           # [256, 1024]
    whT = _trunc10(w2T)
    wlT = np.ascontiguousarray(w2T - whT)
    negwsq = -(weight.astype(np.float64) ** 2).sum(axis=1).astype(np.float32)[None, :]
    nwh = _trunc10(negwsq)
    nwl = np.ascontiguousarray(negwsq - nwh)
    in_maps = []
    for c in range(N_CORES):
        sl = slice(c * TOK_PER_CORE, (c + 1) * TOK_PER_CORE)
        in_maps.append(
            {
                "zhT": np.ascontiguousarray(zhT_full[:, sl]),
                "zlT": np.ascontiguousarray(zlT_full[:, sl]),
                "whT": whT,
                "wlT": wlT,
                "nwh": nwh,
                "nwl": nwl,
                "wsrc": weight,
            }
        )
    return in_maps


def kernel(z: np.ndarray, weight: np.ndarray):
    z = np.ascontiguousarray(z, dtype=np.float32)
    weight = np.ascontiguousarray(weight, dtype=np.float32)
    assert z.shape == (N_TOK, D_DIM) and weight.shape == (K_CODES, D_DIM)

    if "nc" not in _cache:
        _cache["nc"] = _build_bass()
    nc = _cache["nc"]

    from concourse.bass_utils import run_bass_kernel_spmd

    in_maps = _prep_inputs(z, weight)
    res = run_bass_kernel_spmd(nc, in_maps, core_ids=list(range(N_CORES)))
    return _assemble(z, weight, [res.results[c] for c in range(N_CORES)])


def _assemble(z, weight, results):
    z_q = np.concatenate([r["zq"] for r in results], axis=0)
    idx = np.concatenate(
        [r["idxo"].T.reshape(-1) for r in results], axis=0
    ).astype(np.int32)
    # loss = BETA * mean(d_min); d_min[n] = ||z_n||^2 - max_k q[n,k]
    sum_vmax = sum(float(r["vmaxo"].sum(dtype=np.float64)) for r in results)
    sum_zsq = float((z.astype(np.float64) ** 2).sum())
    loss = np.float32(BETA * (sum_zsq - sum_vmax) / (N_TOK * D_DIM))
    # straight-through estimator, matching reference arithmetic exactly
    z_q_st = z + (z_q - z)
    return (loss, z_q_st, idx)


# revision 6
# speedup vs baseline: 1.8480x; 1.4092x over previous
"""EMAVectorQuantizer forward on 8 Trainium2 NeuronCores (Bass/Tile).

Reference computation:
    d[n,k] = ||z_n||^2 + ||w_k||^2 - 2 z_n.w_k          n<65536, k<1024, D=256
    idx[n] = argmin_k d[n,k]   (first occurrence)
    z_q    = w[idx];  loss = 0.25*mean((z_q - z)^2);  z_q_st = z + (z_q - z)

Strategy (data parallel over tokens, 8 cores x 8192 tokens):
  argmin_k d = argmax_k q,  q[n,k] = 2 z_n.w_k - ||w_k||^2  (||z||^2 drops).

  Matmul precision: the PE's fast fp32 mode (float32r) rounds both operands
  to ~10 mantissa bits (TF32-like) but multiplies those exactly.  We split
  each operand at 10 mantissa bits (x = xh + xl, xh = trunc10(x)) and
  compute q = zh.wh + zh.wl + zl.wh (+ split bias rows) — every partial
  product is exact to f32, accumulation is f32 in PSUM, the dropped zl.wl
  term is ~1e-5 absolute.  f32-grade accuracy at 1 cycle/row instead of
  native fp32's 4 cycles/row.

  Per 128-token tile:
    - PE: 16 matmuls (fp32r) -> q in PSUM [128,1024] f32
    - DVE tensor_tensor_scan (op0=max): pm = prefix-max of q -> SBUF;
      pm[:,-1] is the row max.
    - ACT: idx = accum(Sign(tmax - pm)) — counts elements before the first
      occurrence of the max (exact f32 compares, exact tie semantics);
      also copies pm[:,-1] (row max, used for the loss) to an output buffer.
    - GPSIMD indirect DMA: z_q rows gathered from weight in HBM by idx.
  loss: sum d_min = sum z^2 - sum_n max_k q  (host f64 final reduction).
"""

import numpy as np

N_TOK = 65536
K_CODES = 1024
D_DIM = 256
N_CORES = 8
TOK_PER_CORE = N_TOK // N_CORES          # 8192
TILES = TOK_PER_CORE // 128              # 64
BETA = 0.25

_cache = {}


def _install_birpatch():
    """Split multi-wait instructions in the BIR: this walrus build allows only
    one sem-wait per instruction, but Tile's kernel-tail drain carries one
    wait per semaphore lane. Extra waits move to single-wait Drain carriers."""
    if _cache.get("birpatch"):
        return
    _cache["birpatch"] = True
    import json as _json

    import concourse.bass_utils as _bu
    import concourse.bass2jax as _b2j

    def _split_multiwait(bir_bytes):
        m = _json.loads(bir_bytes)
        changed = False
        for fn in m.get("functions", []):
            for bb in fn.get("blocks", []):
                out = []
                for inst in bb.get("instructions", []):
                    si = inst.get("sync_info") or {}
                    waits = si.get("on_wait") or []
                    if len(waits) <= 1:
                        out.append(inst)
                        continue
                    changed = True
                    for i, w in enumerate(waits[:-1]):
                        out.append(
                            {
                                "name": f"{inst['name']}-w{i}",
                                "opcode": "EventSemaphore",
                                "engine": inst["engine"],
                                "ins": [],
                                "outs": [],
                                "debug": inst.get("debug", 0),
                                "sync_info": {"on_update": [], "on_wait": [w]},
                            }
                        )
                    si["on_wait"] = [waits[-1]]
                    inst["sync_info"] = si
                    out.append(inst)
                bb["instructions"] = out
        return _json.dumps(m).encode() if changed else bir_bytes

    orig = _bu.compile_bir_kernel

    def patched(bir_json, tmpdir, neff_name="file.neff"):
        if isinstance(bir_json, str):
            bir_json = bir_json.encode()
        return orig(_split_multiwait(bir_json), tmpdir, neff_name)

    _bu.compile_bir_kernel = patched
    _b2j.compile_bir_kernel = patched


def _build_bass():
    _install_birpatch()
    import concourse.bass as bass
    import concourse.tile as tile
    import concourse.mybir as mybir

    f32 = mybir.dt.float32
    r32 = mybir.dt.float32r
    nc = bass.Bass()

    zhT = nc.dram_tensor("zhT", [D_DIM, TOK_PER_CORE], r32, kind="ExternalInput")
    zlT = nc.dram_tensor("zlT", [D_DIM, TOK_PER_CORE], r32, kind="ExternalInput")
    whT = nc.dram_tensor("whT", [D_DIM, K_CODES], r32, kind="ExternalInput")
    wlT = nc.dram_tensor("wlT", [D_DIM, K_CODES], r32, kind="ExternalInput")
    nwh = nc.dram_tensor("nwh", [1, K_CODES], r32, kind="ExternalInput")
    nwl = nc.dram_tensor("nwl", [1, K_CODES], r32, kind="ExternalInput")
    wsrc = nc.dram_tensor("wsrc", [K_CODES, D_DIM], f32, kind="ExternalInput")

    zq = nc.dram_tensor("zq", [TOK_PER_CORE, D_DIM], f32, kind="ExternalOutput")
    idxo = nc.dram_tensor("idxo", [128, TILES], mybir.dt.int32, kind="ExternalOutput")
    vmaxo = nc.dram_tensor("vmaxo", [128, TILES], f32, kind="ExternalOutput")

    with tile.TileContext(nc) as tc:
        with (
            tc.tile_pool(name="const", bufs=1) as cpool,
            tc.tile_pool(name="zin", bufs=8) as zpool,
            tc.tile_pool(name="psum", bufs=3, space="PSUM") as qpool,
            tc.tile_pool(name="pmbuf", bufs=3) as pmpool,
            tc.tile_pool(name="junk", bufs=2) as jpool,
            tc.tile_pool(name="small", bufs=1) as spool,
            tc.tile_pool(name="gat", bufs=4) as gpool,
        ):
            wha = cpool.tile([128, K_CODES], r32, tag="wha")
            nc.sync.dma_start(wha[:], whT[0:128, :])
            whb = cpool.tile([128, K_CODES], r32, tag="whb")
            nc.sync.dma_start(whb[:], whT[128:256, :])
            wla = cpool.tile([128, K_CODES], r32, tag="wla")
            nc.sync.dma_start(wla[:], wlT[0:128, :])
            wlb = cpool.tile([128, K_CODES], r32, tag="wlb")
            nc.sync.dma_start(wlb[:], wlT[128:256, :])
            nh = cpool.tile([1, K_CODES], r32, tag="nh")
            nc.sync.dma_start(nh[:], nwh[:])
            nl = cpool.tile([1, K_CODES], r32, tag="nl")
            nc.sync.dma_start(nl[:], nwl[:])
            ones_t = cpool.tile([1, 128], f32, tag="ones")
            nc.vector.memset(ones_t[:], 1.0)
            ones = ones_t[:].bitcast(r32)
            zeros = cpool.tile([128, K_CODES], f32, tag="zeros")
            nc.vector.memset(zeros[:], 0.0)

            idxf = spool.tile([128, TILES], f32, tag="idxf")
            idxi = spool.tile([128, TILES], mybir.dt.int32, tag="idxi")
            vmaxb = spool.tile([128, TILES], f32, tag="vmaxb")

            for tt in range(TILES):
                sl = slice(tt * 128, (tt + 1) * 128)
                zh0 = zpool.tile([128, 128], r32, tag="zh0")
                nc.sync.dma_start(zh0[:], zhT[0:128, sl])
                zh1 = zpool.tile([128, 128], r32, tag="zh1")
                nc.sync.dma_start(zh1[:], zhT[128:256, sl])
                zl0 = zpool.tile([128, 128], r32, tag="zl0")
                nc.sync.dma_start(zl0[:], zlT[0:128, sl])
                zl1 = zpool.tile([128, 128], r32, tag="zl1")
                nc.sync.dma_start(zl1[:], zlT[128:256, sl])

                q = qpool.tile([128, K_CODES], f32, tag="q")
                L, R = slice(0, 512), slice(512, 1024)
                mm = nc.tensor.matmul
                # grouped by stationary operand (5 weight loads per tile)
                mm(q[:, L], zh0[:], wha[:, L], start=True, stop=False)
                mm(q[:, R], zh0[:], wha[:, R], start=True, stop=False)
                mm(q[:, L], zh0[:], wla[:, L], start=False, stop=False)
                mm(q[:, R], zh0[:], wla[:, R], start=False, stop=False)
                mm(q[:, L], zh1[:], whb[:, L], start=False, stop=False)
                mm(q[:, R], zh1[:], whb[:, R], start=False, stop=False)
                mm(q[:, L], zh1[:], wlb[:, L], start=False, stop=False)
                mm(q[:, R], zh1[:], wlb[:, R], start=False, stop=False)
                mm(q[:, L], zl0[:], wha[:, L], start=False, stop=False)
                mm(q[:, R], zl0[:], wha[:, R], start=False, stop=False)
                mm(q[:, L], zl1[:], whb[:, L], start=False, stop=False)
                mm(q[:, R], zl1[:], whb[:, R], start=False, stop=False)
                mm(q[:, L], ones[0:1, :], nh[0:1, L], start=False, stop=False)
                mm(q[:, R], ones[0:1, :], nh[0:1, R], start=False, stop=False)
                mm(q[:, L], ones[0:1, :], nl[0:1, L], start=False, stop=True)
                mm(q[:, R], ones[0:1, :], nl[0:1, R], start=False, stop=True)

                pm = pmpool.tile([128, K_CODES], f32, tag="pm")
                nc.vector.tensor_tensor_scan(
                    out=pm[:],
                    data0=q[:],
                    data1=zeros[:],
                    initial=-3.0e38,
                    op0=mybir.AluOpType.max,
                    op1=mybir.AluOpType.bypass,
                )
                # idx = sum_k Sign(tmax - pm[k]) on the scalar engine
                junk = jpool.tile([128, K_CODES], f32, tag="junk")
                nc.scalar.activation(
                    junk[:],
                    pm[:],
                    mybir.ActivationFunctionType.Sign,
                    bias=pm[:, K_CODES - 1 : K_CODES],
                    scale=-1.0,
                    accum_out=idxf[:, tt : tt + 1],
                )
                # row max for the loss
                nc.scalar.copy(vmaxb[:, tt : tt + 1], pm[:, K_CODES - 1 : K_CODES])

            # f32 -> int32 (values in [0, 1023], exact)
            nc.vector.tensor_copy(idxi[:], idxf[:])

            for tt in range(TILES):
                g = gpool.tile([128, D_DIM], f32, tag="g")
                nc.gpsimd.indirect_dma_start(
                    out=g[:],
                    out_offset=None,
                    in_=wsrc[:],
                    in_offset=bass.IndirectOffsetOnAxis(
                        ap=idxi[:, tt : tt + 1], axis=0
                    ),
                )
                nc.sync.dma_start(zq[tt * 128 : (tt + 1) * 128, :], g[:])

            nc.sync.dma_start(idxo[:], idxi[:])
            nc.sync.dma_start(vmaxo[:], vmaxb[:])

    return nc


def _trunc10(x):
    return (x.view(np.int32) & np.int32(~((1 << 13) - 1))).view(np.float32)


def _prep_inputs(z, weight):
    zT = np.ascontiguousarray(z.T)                         # [256, 65536]
    zhT_full = _trunc10(zT)
    zlT_full = zT - zhT_full
    w2T = np.ascontiguousarray((2.0 * weight).T)           # [256, 1024]
    whT = _trunc10(w2T)
    wlT = np.ascontiguousarray(w2T - whT)
    negwsq = -(weight.astype(np.float64) ** 2).sum(axis=1).astype(np.float32)[None, :]
    nwh = _trunc10(negwsq)
    nwl = np.ascontiguousarray(negwsq - nwh)
    in_maps = []
    for c in range(N_CORES):
        sl = slice(c * TOK_PER_CORE, (c + 1) * TOK_PER_CORE)
        in_maps.append(
            {
                "zhT": np.ascontiguousarray(zhT_full[:, sl]),
                "zlT": np.ascontiguousarray(zlT_full[:, sl]),
                "whT": whT,
                "wlT": wlT,
                "nwh": nwh,
                "nwl": nwl,
                "wsrc": weight,
            }
        )
    return in_maps


def kernel(z: np.ndarray, weight: np.ndarray):
    z = np.ascontiguousarray(z, dtype=np.float32)
    weight = np.ascontiguousarray(weight, dtype=np.float32)
    assert z.shape == (N_TOK, D_DIM) and weight.shape == (K_CODES, D_DIM)

    if "nc" not in _cache:
        _cache["nc"] = _build_bass()
    nc = _cache["nc"]

    from concourse.bass_utils import run_bass_kernel_spmd

    in_maps = _prep_inputs(z, weight)
    res = run_bass_kernel_spmd(nc, in_maps, core_ids=list(range(N_CORES)))
    return _assemble(z, weight, [res.results[c] for c in range(N_CORES)])


def _assemble(z, weight, results):
    z_q = np.concatenate([r["zq"] for r in results], axis=0)
    idx = np.concatenate(
        [r["idxo"].T.reshape(-1) for r in results], axis=0
    ).astype(np.int32)
    # loss = BETA * mean(d_min); d_min[n] = ||z_n||^2 - max_k q[n,k]
    sum_vmax = sum(float(r["vmaxo"].sum(dtype=np.float64)) for r in results)
    sum_zsq = float((z.astype(np.float64) ** 2).sum())
    loss = np.float32(BETA * (sum_zsq - sum_vmax) / (N_TOK * D_DIM))
    # straight-through estimator, matching reference arithmetic exactly
    z_q_st = z + (z_q - z)
    return (loss, z_q_st, idx)


# revision 7
# speedup vs baseline: 2.6045x; 1.4093x over previous
"""EMAVectorQuantizer forward on 8 Trainium2 NeuronCores (Bass/Tile).

Reference computation:
    d[n,k] = ||z_n||^2 + ||w_k||^2 - 2 z_n.w_k          n<65536, k<1024, D=256
    idx[n] = argmin_k d[n,k]   (first occurrence)
    z_q    = w[idx];  loss = 0.25*mean((z_q - z)^2);  z_q_st = z + (z_q - z)

Strategy (data parallel over tokens, 8 cores x 8192 tokens):
  argmin_k d = argmax_k q,  q[n,k] = 2 z_n.w_k - ||w_k||^2  (||z||^2 drops).

  Matmul precision: the PE's fast fp32 mode (float32r) rounds both operands
  to ~10 mantissa bits (TF32-like) but multiplies those exactly.  We split
  each operand at 10 mantissa bits (x = xh + xl, xh = trunc10(x)) and
  compute q = zh.wh + zh.wl + zl.wh (+ split bias rows) — every partial
  product is exact to f32, accumulation is f32 in PSUM, the dropped zl.wl
  term is ~1e-5 absolute.  f32-grade accuracy at 1 cycle/row instead of
  native fp32's 4 cycles/row.

  Per 128-token tile:
    - PE: 16 matmuls (fp32r) -> q in PSUM [128,1024] f32
    - DVE tensor_tensor_scan (op0=max): pm = prefix-max of q -> SBUF;
      pm[:,-1] is the row max.
    - ACT: idx = accum(Sign(tmax - pm)) — counts elements before the first
      occurrence of the max (exact f32 compares, exact tie semantics);
      also copies pm[:,-1] (row max, used for the loss) to an output buffer.
    - GPSIMD indirect DMA: z_q rows gathered from weight in HBM by idx.
  loss: sum d_min = sum z^2 - sum_n max_k q  (host f64 final reduction).
"""

import numpy as np

N_TOK = 65536
K_CODES = 1024
D_DIM = 256
N_CORES = 8
TOK_PER_CORE = N_TOK // N_CORES          # 8192
TILES = TOK_PER_CORE // 128              # 64
BETA = 0.25

_cache = {}


def _install_birpatch():
    """Split multi-wait instructions in the BIR: this walrus build allows only
    one sem-wait per instruction, but Tile's kernel-tail drain carries one
    wait per semaphore lane. Extra waits move to single-wait Drain carriers."""
    if _cache.get("birpatch"):
        return
    _cache["birpatch"] = True
    import json as _json

    import concourse.bass_utils as _bu
    import concourse.bass2jax as _b2j

    def _split_multiwait(bir_bytes):
        m = _json.loads(bir_bytes)
        changed = False
        for fn in m.get("functions", []):
            for bb in fn.get("blocks", []):
                out = []
                for inst in bb.get("instructions", []):
                    si = inst.get("sync_info") or {}
                    waits = si.get("on_wait") or []
                    if len(waits) <= 1:
                        out.append(inst)
                        continue
                    changed = True
                    for i, w in enumerate(waits[:-1]):
                        out.append(
                            {
                                "name": f"{inst['name']}-w{i}",
                                "opcode": "EventSemaphore",
                                "engine": inst["engine"],
                                "ins": [],
                                "outs": [],
                                "debug": inst.get("debug", 0),
                                "sync_info": {"on_update": [], "on_wait": [w]},
                            }
                        )
                    si["on_wait"] = [waits[-1]]
                    inst["sync_info"] = si
                    out.append(inst)
                bb["instructions"] = out
        return _json.dumps(m).encode() if changed else bir_bytes

    orig = _bu.compile_bir_kernel

    def patched(bir_json, tmpdir, neff_name="file.neff"):
        if isinstance(bir_json, str):
            bir_json = bir_json.encode()
        return orig(_split_multiwait(bir_json), tmpdir, neff_name)

    _bu.compile_bir_kernel = patched
    _b2j.compile_bir_kernel = patched


def _build_bass():
    _install_birpatch()
    import concourse.bass as bass
    import concourse.tile as tile
    import concourse.mybir as mybir

    f32 = mybir.dt.float32
    r32 = mybir.dt.float32r
    nc = bass.Bass()

    zsp = nc.dram_tensor("zsp", [2 * D_DIM, TOK_PER_CORE], r32, kind="ExternalInput")
    whT = nc.dram_tensor("whT", [D_DIM, K_CODES], r32, kind="ExternalInput")
    wlT = nc.dram_tensor("wlT", [D_DIM, K_CODES], r32, kind="ExternalInput")
    nhl = nc.dram_tensor("nhl", [2, K_CODES], r32, kind="ExternalInput")
    wsrc = nc.dram_tensor("wsrc", [K_CODES, D_DIM], f32, kind="ExternalInput")

    zq = nc.dram_tensor("zq", [TOK_PER_CORE, D_DIM], f32, kind="ExternalOutput")
    idxo = nc.dram_tensor("idxo", [128, TILES], mybir.dt.int32, kind="ExternalOutput")
    vmaxo = nc.dram_tensor("vmaxo", [128, TILES], f32, kind="ExternalOutput")

    with tile.TileContext(nc) as tc:
        with (
            tc.tile_pool(name="const", bufs=1) as cpool,
            tc.tile_pool(name="zin", bufs=8) as zpool,
            tc.tile_pool(name="psum", bufs=3, space="PSUM") as qpool,
            tc.tile_pool(name="pmbuf", bufs=3) as pmpool,
            tc.tile_pool(name="junk", bufs=2) as jpool,
            tc.tile_pool(name="small", bufs=1) as spool,
            tc.tile_pool(name="gat", bufs=4) as gpool,
        ):
            wha = cpool.tile([128, K_CODES], r32, tag="wha")
            nc.sync.dma_start(wha[:], whT[0:128, :])
            whb = cpool.tile([128, K_CODES], r32, tag="whb")
            nc.sync.dma_start(whb[:], whT[128:256, :])
            wla = cpool.tile([128, K_CODES], r32, tag="wla")
            nc.sync.dma_start(wla[:], wlT[0:128, :])
            wlb = cpool.tile([128, K_CODES], r32, tag="wlb")
            nc.sync.dma_start(wlb[:], wlT[128:256, :])
            nhl_t = cpool.tile([2, K_CODES], r32, tag="nhl")
            nc.sync.dma_start(nhl_t[:], nhl[:])
            ones_t = cpool.tile([2, 128], f32, tag="ones")
            nc.vector.memset(ones_t[:], 1.0)
            ones = ones_t[:].bitcast(r32)
            zeros = cpool.tile([128, K_CODES], f32, tag="zeros")
            nc.vector.memset(zeros[:], 0.0)

            idxf = spool.tile([128, TILES], f32, tag="idxf")
            idxi = spool.tile([128, TILES], mybir.dt.int32, tag="idxi")
            vmaxb = spool.tile([128, TILES], f32, tag="vmaxb")

            zsrc = zsp.rearrange("(c p) t -> p c t", p=128)
            for tt in range(TILES):
                zt4 = zpool.tile([128, 512], r32, tag="zt4")
                nc.sync.dma_start(
                    zt4[:].rearrange("p (c j) -> p c j", c=4),
                    zsrc[:, :, tt * 128 : (tt + 1) * 128],
                )
                zh0 = zt4[:, 0:128]
                zh1 = zt4[:, 128:256]
                zl0 = zt4[:, 256:384]
                zl1 = zt4[:, 384:512]

                q = qpool.tile([128, K_CODES], f32, tag="q")
                L, R = slice(0, 512), slice(512, 1024)
                mm = nc.tensor.matmul
                # grouped by stationary operand (5 weight loads per tile)
                mm(q[:, L], zh0, wha[:, L], start=True, stop=False)
                mm(q[:, R], zh0, wha[:, R], start=True, stop=False)
                mm(q[:, L], zh0, wla[:, L], start=False, stop=False)
                mm(q[:, R], zh0, wla[:, R], start=False, stop=False)
                mm(q[:, L], zh1, whb[:, L], start=False, stop=False)
                mm(q[:, R], zh1, whb[:, R], start=False, stop=False)
                mm(q[:, L], zh1, wlb[:, L], start=False, stop=False)
                mm(q[:, R], zh1, wlb[:, R], start=False, stop=False)
                mm(q[:, L], zl0, wha[:, L], start=False, stop=False)
                mm(q[:, R], zl0, wha[:, R], start=False, stop=False)
                mm(q[:, L], zl1, whb[:, L], start=False, stop=False)
                mm(q[:, R], zl1, whb[:, R], start=False, stop=False)
                mm(q[:, L], ones[0:2, :], nhl_t[0:2, L], start=False, stop=True)
                mm(q[:, R], ones[0:2, :], nhl_t[0:2, R], start=False, stop=True)

                pm = pmpool.tile([128, K_CODES], f32, tag="pm")
                nc.vector.tensor_tensor_scan(
                    out=pm[:],
                    data0=q[:],
                    data1=zeros[:],
                    initial=-3.0e38,
                    op0=mybir.AluOpType.max,
                    op1=mybir.AluOpType.bypass,
                )
                # idx = sum_k Sign(tmax - pm[k]) on the scalar engine
                junk = jpool.tile([128, K_CODES], f32, tag="junk")
                nc.scalar.activation(
                    junk[:],
                    pm[:],
                    mybir.ActivationFunctionType.Sign,
                    bias=pm[:, K_CODES - 1 : K_CODES],
                    scale=-1.0,
                    accum_out=idxf[:, tt : tt + 1],
                )
                # row max for the loss
                nc.scalar.copy(vmaxb[:, tt : tt + 1], pm[:, K_CODES - 1 : K_CODES])

                # f32 -> int32 cast and z_q gather, pipelined per tile
                nc.vector.tensor_copy(idxi[:, tt : tt + 1], idxf[:, tt : tt + 1])
                g = gpool.tile([128, D_DIM], f32, tag="g")
                nc.gpsimd.indirect_dma_start(
                    out=g[:],
                    out_offset=None,
                    in_=wsrc[:],
                    in_offset=bass.IndirectOffsetOnAxis(
                        ap=idxi[:, tt : tt + 1], axis=0
                    ),
                )
                nc.sync.dma_start(zq[tt * 128 : (tt + 1) * 128, :], g[:])

            nc.sync.dma_start(idxo[:], idxi[:])
            nc.sync.dma_start(vmaxo[:], vmaxb[:])

    return nc


def _trunc10(x):
    return (x.view(np.int32) & np.int32(~((1 << 13) - 1))).view(np.float32)


def _prep_inputs(z, weight):
    zT = np.ascontiguousarray(z.T)                         # [256, 65536]
    zhT_full = _trunc10(zT)
    zlT_full = zT - zhT_full
    zsp_full = np.concatenate([zhT_full, zlT_full], axis=0)  # [512, 65536]
    w2T = np.ascontiguousarray((2.0 * weight).T)           # [256, 1024]
    whT = _trunc10(w2T)
    wlT = np.ascontiguousarray(w2T - whT)
    negwsq = -(weight.astype(np.float64) ** 2).sum(axis=1).astype(np.float32)[None, :]
    nwh = _trunc10(negwsq)
    nhl = np.ascontiguousarray(np.concatenate([nwh, negwsq - nwh], axis=0))
    in_maps = []
    for c in range(N_CORES):
        sl = slice(c * TOK_PER_CORE, (c + 1) * TOK_PER_CORE)
        in_maps.append(
            {
                "zsp": np.ascontiguousarray(zsp_full[:, sl]),
                "whT": whT,
                "wlT": wlT,
                "nhl": nhl,
                "wsrc": weight,
            }
        )
    return in_maps


def kernel(z: np.ndarray, weight: np.ndarray):
    z = np.ascontiguousarray(z, dtype=np.float32)
    weight = np.ascontiguousarray(weight, dtype=np.float32)
    assert z.shape == (N_TOK, D_DIM) and weight.shape == (K_CODES, D_DIM)

    if "nc" not in _cache:
        _cache["nc"] = _build_bass()
    nc = _cache["nc"]

    from concourse.bass_utils import run_bass_kernel_spmd

    in_maps = _prep_inputs(z, weight)
    res = run_bass_kernel_spmd(nc, in_maps, core_ids=list(range(N_CORES)))
    return _assemble(z, weight, [res.results[c] for c in range(N_CORES)])


def _assemble(z, weight, results):
    z_q = np.concatenate([r["zq"] for r in results], axis=0)
    idx = np.concatenate(
        [r["idxo"].T.reshape(-1) for r in results], axis=0
    ).astype(np.int32)
    # loss = BETA * mean(d_min); d_min[n] = ||z_n||^2 - max_k q[n,k]
    sum_vmax = sum(float(r["vmaxo"].sum(dtype=np.float64)) for r in results)
    sum_zsq = float((z.astype(np.float64) ** 2).sum())
    loss = np.float32(BETA * (sum_zsq - sum_vmax) / (N_TOK * D_DIM))
    # straight-through estimator, matching reference arithmetic exactly
    z_q_st = z + (z_q - z)
    return (loss, z_q_st, idx)


# revision 8
# speedup vs baseline: 2.7116x; 1.0411x over previous
"""EMAVectorQuantizer forward on 8 Trainium2 NeuronCores (Bass/Tile).

Reference computation:
    d[n,k] = ||z_n||^2 + ||w_k||^2 - 2 z_n.w_k          n<65536, k<1024, D=256
    idx[n] = argmin_k d[n,k]   (first occurrence)
    z_q    = w[idx];  loss = 0.25*mean((z_q - z)^2);  z_q_st = z + (z_q - z)

Strategy (data parallel over tokens, 8 cores x 8192 tokens):
  argmin_k d = argmax_k q,  q[n,k] = 2 z_n.w_k - ||w_k||^2  (||z||^2 drops).

  Matmul precision: the PE's fast fp32 mode (float32r) rounds both operands
  to ~10 mantissa bits (TF32-like) but multiplies those exactly.  We split
  each operand at 10 mantissa bits (x = xh + xl, xh = trunc10(x)) and
  compute q = zh.wh + zh.wl + zl.wh (+ split bias rows) — every partial
  product is exact to f32, accumulation is f32 in PSUM, the dropped zl.wl
  term is ~1e-5 absolute.  f32-grade accuracy at 1 cycle/row instead of
  native fp32's 4 cycles/row.

  Per 128-token tile:
    - PE: 16 matmuls (fp32r) -> q in PSUM [128,1024] f32
    - DVE tensor_tensor_scan (op0=max): pm = prefix-max of q -> SBUF;
      pm[:,-1] is the row max.
    - ACT: idx = accum(Sign(tmax - pm)) — counts elements before the first
      occurrence of the max (exact f32 compares, exact tie semantics);
      also copies pm[:,-1] (row max, used for the loss) to an output buffer.
    - GPSIMD indirect DMA: z_q rows gathered from weight in HBM by idx.
  loss: sum d_min = sum z^2 - sum_n max_k q  (host f64 final reduction).
"""

import numpy as np

N_TOK = 65536
K_CODES = 1024
D_DIM = 256
N_CORES = 8
TOK_PER_CORE = N_TOK // N_CORES          # 8192
TILES = TOK_PER_CORE // 128              # 64
BETA = 0.25

_cache = {}


def _install_birpatch():
    """Split multi-wait instructions in the BIR: this walrus build allows only
    one sem-wait per instruction, but Tile's kernel-tail drain carries one
    wait per semaphore lane. Extra waits move to single-wait Drain carriers."""
    if _cache.get("birpatch"):
        return
    _cache["birpatch"] = True
    import json as _json

    import concourse.bass_utils as _bu
    import concourse.bass2jax as _b2j

    def _split_multiwait(bir_bytes):
        m = _json.loads(bir_bytes)
        changed = False
        for fn in m.get("functions", []):
            for bb in fn.get("blocks", []):
                out = []
                for inst in bb.get("instructions", []):
                    si = inst.get("sync_info") or {}
                    waits = si.get("on_wait") or []
                    if len(waits) <= 1:
                        out.append(inst)
                        continue
                    changed = True
                    for i, w in enumerate(waits[:-1]):
                        out.append(
                            {
                                "name": f"{inst['name']}-w{i}",
                                "opcode": "EventSemaphore",
                                "engine": inst["engine"],
                                "ins": [],
                                "outs": [],
                                "debug": inst.get("debug", 0),
                                "sync_info": {"on_update": [], "on_wait": [w]},
                            }
                        )
                    si["on_wait"] = [waits[-1]]
                    inst["sync_info"] = si
                    out.append(inst)
                bb["instructions"] = out
        return _json.dumps(m).encode() if changed else bir_bytes

    orig = _bu.compile_bir_kernel

    def patched(bir_json, tmpdir, neff_name="file.neff"):
        if isinstance(bir_json, str):
            bir_json = bir_json.encode()
        return orig(_split_multiwait(bir_json), tmpdir, neff_name)

    _bu.compile_bir_kernel = patched
    _b2j.compile_bir_kernel = patched


def _build_bass():
    _install_birpatch()
    import concourse.bass as bass
    import concourse.tile as tile
    import concourse.mybir as mybir

    f32 = mybir.dt.float32
    r32 = mybir.dt.float32r
    nc = bass.Bass()

    zsp = nc.dram_tensor("zsp", [2 * D_DIM, TOK_PER_CORE], r32, kind="ExternalInput")
    whT = nc.dram_tensor("whT", [D_DIM, K_CODES], r32, kind="ExternalInput")
    wlT = nc.dram_tensor("wlT", [D_DIM, K_CODES], r32, kind="ExternalInput")
    nhl = nc.dram_tensor("nhl", [2, K_CODES], r32, kind="ExternalInput")
    wsrc = nc.dram_tensor("wsrc", [K_CODES, D_DIM], f32, kind="ExternalInput")

    zq = nc.dram_tensor("zq", [TOK_PER_CORE, D_DIM], f32, kind="ExternalOutput")
    idxo = nc.dram_tensor("idxo", [128, TILES], mybir.dt.int32, kind="ExternalOutput")
    vmaxo = nc.dram_tensor("vmaxo", [128, TILES], f32, kind="ExternalOutput")

    with tile.TileContext(nc) as tc:
        with (
            tc.tile_pool(name="const", bufs=1) as cpool,
            tc.tile_pool(name="zin", bufs=8) as zpool,
            tc.tile_pool(name="psum", bufs=3, space="PSUM") as qpool,
            tc.tile_pool(name="pmbuf", bufs=3) as pmpool,
            tc.tile_pool(name="junk", bufs=2) as jpool,
            tc.tile_pool(name="small", bufs=1) as spool,
            tc.tile_pool(name="gat", bufs=4) as gpool,
        ):
            wha = cpool.tile([128, K_CODES], r32, tag="wha")
            nc.sync.dma_start(wha[:], whT[0:128, :])
            whb = cpool.tile([128, K_CODES], r32, tag="whb")
            nc.sync.dma_start(whb[:], whT[128:256, :])
            wla = cpool.tile([128, K_CODES], r32, tag="wla")
            nc.sync.dma_start(wla[:], wlT[0:128, :])
            wlb = cpool.tile([128, K_CODES], r32, tag="wlb")
            nc.sync.dma_start(wlb[:], wlT[128:256, :])
            nhl_t = cpool.tile([2, K_CODES], r32, tag="nhl")
            nc.sync.dma_start(nhl_t[:], nhl[:])
            ones_t = cpool.tile([2, 128], f32, tag="ones")
            nc.vector.memset(ones_t[:], 1.0)
            ones = ones_t[:].bitcast(r32)
            zeros = cpool.tile([128, K_CODES], f32, tag="zeros")
            nc.vector.memset(zeros[:], 0.0)

            idxf = spool.tile([128, TILES], f32, tag="idxf")
            idxi = spool.tile([128, TILES], mybir.dt.int32, tag="idxi")
            vmaxb = spool.tile([128, TILES], f32, tag="vmaxb")

            zsrc = zsp.rearrange("(c p) t -> p c t", p=128)
            for tt in range(TILES):
                zt4 = zpool.tile([128, 512], r32, tag="zt4")
                nc.sync.dma_start(
                    zt4[:].rearrange("p (c j) -> p c j", c=4),
                    zsrc[:, :, tt * 128 : (tt + 1) * 128],
                )
                zh0 = zt4[:, 0:128]
                zh1 = zt4[:, 128:256]
                zl0 = zt4[:, 256:384]
                zl1 = zt4[:, 384:512]

                q = qpool.tile([128, K_CODES], f32, tag="q")
                L, R = slice(0, 512), slice(512, 1024)
                mm = nc.tensor.matmul
                # grouped by stationary operand (5 weight loads per tile)
                mm(q[:, L], zh0, wha[:, L], start=True, stop=False)
                mm(q[:, R], zh0, wha[:, R], start=True, stop=False)
                mm(q[:, L], zh0, wla[:, L], start=False, stop=False)
                mm(q[:, R], zh0, wla[:, R], start=False, stop=False)
                mm(q[:, L], zh1, whb[:, L], start=False, stop=False)
                mm(q[:, R], zh1, whb[:, R], start=False, stop=False)
                mm(q[:, L], zh1, wlb[:, L], start=False, stop=False)
                mm(q[:, R], zh1, wlb[:, R], start=False, stop=False)
                mm(q[:, L], zl0, wha[:, L], start=False, stop=False)
                mm(q[:, R], zl0, wha[:, R], start=False, stop=False)
                mm(q[:, L], zl1, whb[:, L], start=False, stop=False)
                mm(q[:, R], zl1, whb[:, R], start=False, stop=False)
                mm(q[:, L], ones[0:2, :], nhl_t[0:2, L], start=False, stop=True)
                mm(q[:, R], ones[0:2, :], nhl_t[0:2, R], start=False, stop=True)

                pm = pmpool.tile([128, K_CODES], f32, tag="pm")
                nc.vector.tensor_tensor_scan(
                    out=pm[:],
                    data0=q[:],
                    data1=zeros[:],
                    initial=-3.0e38,
                    op0=mybir.AluOpType.max,
                    op1=mybir.AluOpType.bypass,
                )
                # idx = sum_k Sign(tmax - pm[k]) on the scalar engine
                junk = jpool.tile([128, K_CODES], f32, tag="junk")
                nc.scalar.activation(
                    junk[:],
                    pm[:],
                    mybir.ActivationFunctionType.Sign,
                    bias=pm[:, K_CODES - 1 : K_CODES],
                    scale=-1.0,
                    accum_out=idxf[:, tt : tt + 1],
                )
                # row max for the loss
                nc.scalar.copy(vmaxb[:, tt : tt + 1], pm[:, K_CODES - 1 : K_CODES])

                # f32 -> int32 cast and z_q gather, pipelined per tile
                # (cast on gpsimd: keeps DVE free for the scans and chains
                # naturally into the gpsimd-issued indirect DMA)
                nc.gpsimd.tensor_copy(idxi[:, tt : tt + 1], idxf[:, tt : tt + 1])
                g = gpool.tile([128, D_DIM], f32, tag="g")
                nc.gpsimd.indirect_dma_start(
                    out=g[:],
                    out_offset=None,
                    in_=wsrc[:],
                    in_offset=bass.IndirectOffsetOnAxis(
                        ap=idxi[:, tt : tt + 1], axis=0
                    ),
                )
                nc.sync.dma_start(zq[tt * 128 : (tt + 1) * 128, :], g[:])

            nc.sync.dma_start(idxo[:], idxi[:])
            nc.sync.dma_start(vmaxo[:], vmaxb[:])

    return nc


def _trunc10(x):
    return (x.view(np.int32) & np.int32(~((1 << 13) - 1))).view(np.float32)


def _prep_inputs(z, weight):
    zT = np.ascontiguousarray(z.T)                         # [256, 65536]
    zhT_full = _trunc10(zT)
    zlT_full = zT - zhT_full
    zsp_full = np.concatenate([zhT_full, zlT_full], axis=0)  # [512, 65536]
    w2T = np.ascontiguousarray((2.0 * weight).T)           # [256, 1024]
    whT = _trunc10(w2T)
    wlT = np.ascontiguousarray(w2T - whT)
    negwsq = -(weight.astype(np.float64) ** 2).sum(axis=1).astype(np.float32)[None, :]
    nwh = _trunc10(negwsq)
    nhl = np.ascontiguousarray(np.concatenate([nwh, negwsq - nwh], axis=0))
    in_maps = []
    for c in range(N_CORES):
        sl = slice(c * TOK_PER_CORE, (c + 1) * TOK_PER_CORE)
        in_maps.append(
            {
                "zsp": np.ascontiguousarray(zsp_full[:, sl]),
                "whT": whT,
                "wlT": wlT,
                "nhl": nhl,
                "wsrc": weight,
            }
        )
    return in_maps


def kernel(z: np.ndarray, weight: np.ndarray):
    z = np.ascontiguousarray(z, dtype=np.float32)
    weight = np.ascontiguousarray(weight, dtype=np.float32)
    assert z.shape == (N_TOK, D_DIM) and weight.shape == (K_CODES, D_DIM)

    if "nc" not in _cache:
        _cache["nc"] = _build_bass()
    nc = _cache["nc"]

    from concourse.bass_utils import run_bass_kernel_spmd

    in_maps = _prep_inputs(z, weight)
    res = run_bass_kernel_spmd(nc, in_maps, core_ids=list(range(N_CORES)))
    return _assemble(z, weight, [res.results[c] for c in range(N_CORES)])


def _assemble(z, weight, results):
    z_q = np.concatenate([r["zq"] for r in results], axis=0)
    idx = np.concatenate(
        [r["idxo"].T.reshape(-1) for r in results], axis=0
    ).astype(np.int32)
    # loss = BETA * mean(d_min); d_min[n] = ||z_n||^2 - max_k q[n,k]
    sum_vmax = sum(float(r["vmaxo"].sum(dtype=np.float64)) for r in results)
    sum_zsq = float((z.astype(np.float64) ** 2).sum())
    loss = np.float32(BETA * (sum_zsq - sum_vmax) / (N_TOK * D_DIM))
    # straight-through estimator, matching reference arithmetic exactly
    z_q_st = z + (z_q - z)
    return (loss, z_q_st, idx)


# revision 9
# speedup vs baseline: 2.7392x; 1.0102x over previous
"""EMAVectorQuantizer forward on 8 Trainium2 NeuronCores (Bass/Tile).

Reference computation:
    d[n,k] = ||z_n||^2 + ||w_k||^2 - 2 z_n.w_k          n<65536, k<1024, D=256
    idx[n] = argmin_k d[n,k]   (first occurrence)
    z_q    = w[idx];  loss = 0.25*mean((z_q - z)^2);  z_q_st = z + (z_q - z)

Strategy (data parallel over tokens, 8 cores x 8192 tokens):
  argmin_k d = argmax_k q,  q[n,k] = 2 z_n.w_k - ||w_k||^2  (||z||^2 drops).

  Matmul precision: the PE's fast fp32 mode (float32r) rounds both operands
  to ~10 mantissa bits (TF32-like) but multiplies those exactly.  We split
  each operand at 10 mantissa bits (x = xh + xl, xh = trunc10(x)) and
  compute q = zh.wh + zh.wl + zl.wh (+ split bias rows) — every partial
  product is exact to f32, accumulation is f32 in PSUM, the dropped zl.wl
  term is ~1e-5 absolute.  f32-grade accuracy at 1 cycle/row instead of
  native fp32's 4 cycles/row.

  Per 128-token tile:
    - PE: 16 matmuls (fp32r) -> q in PSUM [128,1024] f32
    - DVE tensor_tensor_scan (op0=max): pm = prefix-max of q -> SBUF;
      pm[:,-1] is the row max.
    - ACT: idx = accum(Sign(tmax - pm)) — counts elements before the first
      occurrence of the max (exact f32 compares, exact tie semantics);
      also copies pm[:,-1] (row max, used for the loss) to an output buffer.
    - GPSIMD indirect DMA: z_q rows gathered from weight in HBM by idx.
  loss: sum d_min = sum z^2 - sum_n max_k q  (host f64 final reduction).
"""

import numpy as np

N_TOK = 65536
K_CODES = 1024
D_DIM = 256
N_CORES = 8
TOK_PER_CORE = N_TOK // N_CORES          # 8192
TILES = TOK_PER_CORE // 128              # 64
BETA = 0.25

_cache = {}


def _install_birpatch():
    """Split multi-wait instructions in the BIR: this walrus build allows only
    one sem-wait per instruction, but Tile's kernel-tail drain carries one
    wait per semaphore lane. Extra waits move to single-wait Drain carriers."""
    if _cache.get("birpatch"):
        return
    _cache["birpatch"] = True
    import json as _json

    import concourse.bass_utils as _bu
    import concourse.bass2jax as _b2j

    def _split_multiwait(bir_bytes):
        m = _json.loads(bir_bytes)
        changed = False
        for fn in m.get("functions", []):
            for bb in fn.get("blocks", []):
                out = []
                for inst in bb.get("instructions", []):
                    si = inst.get("sync_info") or {}
                    waits = si.get("on_wait") or []
                    if len(waits) <= 1:
                        out.append(inst)
                        continue
                    changed = True
                    for i, w in enumerate(waits[:-1]):
                        out.append(
                            {
                                "name": f"{inst['name']}-w{i}",
                                "opcode": "EventSemaphore",
                                "engine": inst["engine"],
                                "ins": [],
                                "outs": [],
                                "debug": inst.get("debug", 0),
                                "sync_info": {"on_update": [], "on_wait": [w]},
                            }
                        )
                    si["on_wait"] = [waits[-1]]
                    inst["sync_info"] = si
                    out.append(inst)
                bb["instructions"] = out
        return _json.dumps(m).encode() if changed else bir_bytes

    orig = _bu.compile_bir_kernel

    def patched(bir_json, tmpdir, neff_name="file.neff"):
        if isinstance(bir_json, str):
            bir_json = bir_json.encode()
        return orig(_split_multiwait(bir_json), tmpdir, neff_name)

    _bu.compile_bir_kernel = patched
    _b2j.compile_bir_kernel = patched


def _build_bass():
    _install_birpatch()
    import concourse.bass as bass
    import concourse.tile as tile
    import concourse.mybir as mybir

    f32 = mybir.dt.float32
    r32 = mybir.dt.float32r
    nc = bass.Bass()

    zsp = nc.dram_tensor("zsp", [2 * D_DIM, TOK_PER_CORE], r32, kind="ExternalInput")
    whT = nc.dram_tensor("whT", [D_DIM, K_CODES], r32, kind="ExternalInput")
    wlT = nc.dram_tensor("wlT", [D_DIM, K_CODES], r32, kind="ExternalInput")
    nhl = nc.dram_tensor("nhl", [2, K_CODES], r32, kind="ExternalInput")
    wsrc = nc.dram_tensor("wsrc", [K_CODES, D_DIM], f32, kind="ExternalInput")

    zq = nc.dram_tensor("zq", [TOK_PER_CORE, D_DIM], f32, kind="ExternalOutput")
    idxo = nc.dram_tensor("idxo", [128, TILES], mybir.dt.int32, kind="ExternalOutput")
    vmaxo = nc.dram_tensor("vmaxo", [128, TILES], f32, kind="ExternalOutput")

    with tile.TileContext(nc) as tc:
        with (
            tc.tile_pool(name="const", bufs=1) as cpool,
            tc.tile_pool(name="zin", bufs=8) as zpool,
            tc.tile_pool(name="psum", bufs=4, space="PSUM") as qpool,
            tc.tile_pool(name="pmbuf", bufs=3) as pmpool,
            tc.tile_pool(name="junk", bufs=2) as jpool,
            tc.tile_pool(name="small", bufs=1) as spool,
            tc.tile_pool(name="gat", bufs=4) as gpool,
        ):
            wha = cpool.tile([128, K_CODES], r32, tag="wha")
            nc.sync.dma_start(wha[:], whT[0:128, :])
            whb = cpool.tile([128, K_CODES], r32, tag="whb")
            nc.sync.dma_start(whb[:], whT[128:256, :])
            wla = cpool.tile([128, K_CODES], r32, tag="wla")
            nc.sync.dma_start(wla[:], wlT[0:128, :])
            wlb = cpool.tile([128, K_CODES], r32, tag="wlb")
            nc.sync.dma_start(wlb[:], wlT[128:256, :])
            nhl_t = cpool.tile([2, K_CODES], r32, tag="nhl")
            nc.sync.dma_start(nhl_t[:], nhl[:])
            ones_t = cpool.tile([2, 128], f32, tag="ones")
            nc.vector.memset(ones_t[:], 1.0)
            ones = ones_t[:].bitcast(r32)
            zeros = cpool.tile([128, K_CODES], f32, tag="zeros")
            nc.vector.memset(zeros[:], 0.0)

            idxf = spool.tile([128, TILES], f32, tag="idxf")
            idxi = spool.tile([128, TILES], mybir.dt.int32, tag="idxi")
            vmaxb = spool.tile([128, TILES], f32, tag="vmaxb")

            zsrc = zsp.rearrange("(c p) t -> p c t", p=128)
            for tt in range(TILES):
                zt4 = zpool.tile([128, 512], r32, tag="zt4")
                nc.sync.dma_start(
                    zt4[:].rearrange("p (c j) -> p c j", c=4),
                    zsrc[:, :, tt * 128 : (tt + 1) * 128],
                )
                zh0 = zt4[:, 0:128]
                zh1 = zt4[:, 128:256]
                zl0 = zt4[:, 256:384]
                zl1 = zt4[:, 384:512]

                q = qpool.tile([128, K_CODES], f32, tag="q")
                L, R = slice(0, 512), slice(512, 1024)
                mm = nc.tensor.matmul
                # grouped by stationary operand (5 weight loads per tile)
                mm(q[:, L], zh0, wha[:, L], start=True, stop=False)
                mm(q[:, R], zh0, wha[:, R], start=True, stop=False)
                mm(q[:, L], zh0, wla[:, L], start=False, stop=False)
                mm(q[:, R], zh0, wla[:, R], start=False, stop=False)
                mm(q[:, L], zh1, whb[:, L], start=False, stop=False)
                mm(q[:, R], zh1, whb[:, R], start=False, stop=False)
                mm(q[:, L], zh1, wlb[:, L], start=False, stop=False)
                mm(q[:, R], zh1, wlb[:, R], start=False, stop=False)
                mm(q[:, L], zl0, wha[:, L], start=False, stop=False)
                mm(q[:, R], zl0, wha[:, R], start=False, stop=False)
                mm(q[:, L], zl1, whb[:, L], start=False, stop=False)
                mm(q[:, R], zl1, whb[:, R], start=False, stop=False)
                mm(q[:, L], ones[0:2, :], nhl_t[0:2, L], start=False, stop=True)
                mm(q[:, R], ones[0:2, :], nhl_t[0:2, R], start=False, stop=True)

                pm = pmpool.tile([128, K_CODES], f32, tag="pm")
                nc.vector.tensor_tensor_scan(
                    out=pm[:],
                    data0=q[:],
                    data1=zeros[:],
                    initial=-3.0e38,
                    op0=mybir.AluOpType.max,
                    op1=mybir.AluOpType.bypass,
                )
                # idx = sum_k Sign(tmax - pm[k]) on the scalar engine
                junk = jpool.tile([128, K_CODES], f32, tag="junk")
                nc.scalar.activation(
                    junk[:],
                    pm[:],
                    mybir.ActivationFunctionType.Sign,
                    bias=pm[:, K_CODES - 1 : K_CODES],
                    scale=-1.0,
                    accum_out=idxf[:, tt : tt + 1],
                )
                # row max for the loss
                nc.scalar.copy(vmaxb[:, tt : tt + 1], pm[:, K_CODES - 1 : K_CODES])

                # f32 -> int32 cast and z_q gather, pipelined per tile
                # (cast on gpsimd: keeps DVE free for the scans and chains
                # naturally into the gpsimd-issued indirect DMA)
                nc.gpsimd.tensor_copy(idxi[:, tt : tt + 1], idxf[:, tt : tt + 1])
                g = gpool.tile([128, D_DIM], f32, tag="g")
                nc.gpsimd.indirect_dma_start(
                    out=g[:],
                    out_offset=None,
                    in_=wsrc[:],
                    in_offset=bass.IndirectOffsetOnAxis(
                        ap=idxi[:, tt : tt + 1], axis=0
                    ),
                )
                nc.sync.dma_start(zq[tt * 128 : (tt + 1) * 128, :], g[:])

            nc.sync.dma_start(idxo[:], idxi[:])
            nc.sync.dma_start(vmaxo[:], vmaxb[:])

    return nc


def _trunc10(x):
    return (x.view(np.int32) & np.int32(~((1 << 13) - 1))).view(np.float32)


def _prep_inputs(z, weight):
    zT = np.ascontiguousarray(z.T)                         # [256, 65536]
    zhT_full = _trunc10(zT)
    zlT_full = zT - zhT_full
    zsp_full = np.concatenate([zhT_full, zlT_full], axis=0)  # [512, 65536]
    w2T = np.ascontiguousarray((2.0 * weight).T)           # [256, 1024]
    whT = _trunc10(w2T)
    wlT = np.ascontiguousarray(w2T - whT)
    negwsq = -(weight.astype(np.float64) ** 2).sum(axis=1).astype(np.float32)[None, :]
    nwh = _trunc10(negwsq)
    nhl = np.ascontiguousarray(np.concatenate([nwh, negwsq - nwh], axis=0))
    in_maps = []
    for c in range(N_CORES):
        sl = slice(c * TOK_PER_CORE, (c + 1) * TOK_PER_CORE)
        in_maps.append(
            {
                "zsp": np.ascontiguousarray(zsp_full[:, sl]),
                "whT": whT,
                "wlT": wlT,
                "nhl": nhl,
                "wsrc": weight,
            }
        )
    return in_maps


def kernel(z: np.ndarray, weight: np.ndarray):
    z = np.ascontiguousarray(z, dtype=np.float32)
    weight = np.ascontiguousarray(weight, dtype=np.float32)
    assert z.shape == (N_TOK, D_DIM) and weight.shape == (K_CODES, D_DIM)

    if "nc" not in _cache:
        _cache["nc"] = _build_bass()
    nc = _cache["nc"]

    from concourse.bass_utils import run_bass_kernel_spmd

    in_maps = _prep_inputs(z, weight)
    res = run_bass_kernel_spmd(nc, in_maps, core_ids=list(range(N_CORES)))
    return _assemble(z, weight, [res.results[c] for c in range(N_CORES)])


def _assemble(z, weight, results):
    z_q = np.concatenate([r["zq"] for r in results], axis=0)
    idx = np.concatenate(
        [r["idxo"].T.reshape(-1) for r in results], axis=0
    ).astype(np.int32)
    # loss = BETA * mean(d_min); d_min[n] = ||z_n||^2 - max_k q[n,k]
    sum_vmax = sum(float(r["vmaxo"].sum(dtype=np.float64)) for r in results)
    sum_zsq = float((z.astype(np.float64) ** 2).sum())
    loss = np.float32(BETA * (sum_zsq - sum_vmax) / (N_TOK * D_DIM))
    # straight-through estimator, matching reference arithmetic exactly
    z_q_st = z + (z_q - z)
    return (loss, z_q_st, idx)


# revision 10
# speedup vs baseline: 2.7530x; 1.0050x over previous
"""EMAVectorQuantizer forward on 8 Trainium2 NeuronCores (Bass/Tile).

Reference computation:
    d[n,k] = ||z_n||^2 + ||w_k||^2 - 2 z_n.w_k          n<65536, k<1024, D=256
    idx[n] = argmin_k d[n,k]   (first occurrence)
    z_q    = w[idx];  loss = 0.25*mean((z_q - z)^2);  z_q_st = z + (z_q - z)

Strategy (data parallel over tokens, 8 cores x 8192 tokens):
  argmin_k d = argmax_k q,  q[n,k] = 2 z_n.w_k - ||w_k||^2  (||z||^2 drops).

  Matmul precision: the PE's fast fp32 mode (float32r) rounds both operands
  to ~10 mantissa bits (TF32-like) but multiplies those exactly.  We split
  each operand at 10 mantissa bits (x = xh + xl, xh = trunc10(x)) and
  compute q = zh.wh + zh.wl + zl.wh (+ split bias rows) — every partial
  product is exact to f32, accumulation is f32 in PSUM, the dropped zl.wl
  term is ~1e-5 absolute.  f32-grade accuracy at 1 cycle/row instead of
  native fp32's 4 cycles/row.

  Per 128-token tile:
    - PE: 16 matmuls (fp32r) -> q in PSUM [128,1024] f32
    - DVE tensor_tensor_scan (op0=max): pm = prefix-max of q -> SBUF;
      pm[:,-1] is the row max.
    - ACT: idx = accum(Sign(tmax - pm)) — counts elements before the first
      occurrence of the max (exact f32 compares, exact tie semantics);
      also copies pm[:,-1] (row max, used for the loss) to an output buffer.
    - GPSIMD indirect DMA: z_q rows gathered from weight in HBM by idx.
  loss: sum d_min = sum z^2 - sum_n max_k q  (host f64 final reduction).
"""

import numpy as np

N_TOK = 65536
K_CODES = 1024
D_DIM = 256
N_CORES = 8
TOK_PER_CORE = N_TOK // N_CORES          # 8192
TILES = TOK_PER_CORE // 128              # 64
BETA = 0.25

_cache = {}


def _install_birpatch():
    """Split multi-wait instructions in the BIR: this walrus build allows only
    one sem-wait per instruction, but Tile's kernel-tail drain carries one
    wait per semaphore lane. Extra waits move to single-wait Drain carriers."""
    if _cache.get("birpatch"):
        return
    _cache["birpatch"] = True
    import json as _json

    import concourse.bass_utils as _bu
    import concourse.bass2jax as _b2j

    def _split_multiwait(bir_bytes):
        m = _json.loads(bir_bytes)
        changed = False
        for fn in m.get("functions", []):
            for bb in fn.get("blocks", []):
                out = []
                for inst in bb.get("instructions", []):
                    si = inst.get("sync_info") or {}
                    waits = si.get("on_wait") or []
                    if len(waits) <= 1:
                        out.append(inst)
                        continue
                    changed = True
                    for i, w in enumerate(waits[:-1]):
                        out.append(
                            {
                                "name": f"{inst['name']}-w{i}",
                                "opcode": "EventSemaphore",
                                "engine": inst["engine"],
                                "ins": [],
                                "outs": [],
                                "debug": inst.get("debug", 0),
                                "sync_info": {"on_update": [], "on_wait": [w]},
                            }
                        )
                    si["on_wait"] = [waits[-1]]
                    inst["sync_info"] = si
                    out.append(inst)
                bb["instructions"] = out
        return _json.dumps(m).encode() if changed else bir_bytes

    orig = _bu.compile_bir_kernel

    def patched(bir_json, tmpdir, neff_name="file.neff"):
        if isinstance(bir_json, str):
            bir_json = bir_json.encode()
        return orig(_split_multiwait(bir_json), tmpdir, neff_name)

    _bu.compile_bir_kernel = patched
    _b2j.compile_bir_kernel = patched


def _build_bass():
    _install_birpatch()
    import concourse.bass as bass
    import concourse.tile as tile
    import concourse.mybir as mybir

    f32 = mybir.dt.float32
    r32 = mybir.dt.float32r
    nc = bass.Bass()

    zsp = nc.dram_tensor("zsp", [2 * D_DIM, TOK_PER_CORE], r32, kind="ExternalInput")
    whT = nc.dram_tensor("whT", [D_DIM, K_CODES], r32, kind="ExternalInput")
    wlT = nc.dram_tensor("wlT", [D_DIM, K_CODES], r32, kind="ExternalInput")
    nhl = nc.dram_tensor("nhl", [2, K_CODES], r32, kind="ExternalInput")
    wsrc = nc.dram_tensor("wsrc", [K_CODES, D_DIM], f32, kind="ExternalInput")

    zq = nc.dram_tensor("zq", [TOK_PER_CORE, D_DIM], f32, kind="ExternalOutput")
    idxo = nc.dram_tensor("idxo", [128, TILES], mybir.dt.int32, kind="ExternalOutput")
    vmaxo = nc.dram_tensor("vmaxo", [128, TILES], f32, kind="ExternalOutput")

    with tile.TileContext(nc) as tc:
        with (
            tc.tile_pool(name="const", bufs=1) as cpool,
            tc.tile_pool(name="zin", bufs=8) as zpool,
            tc.tile_pool(name="psum", bufs=4, space="PSUM") as qpool,
            tc.tile_pool(name="pmbuf", bufs=3) as pmpool,
            tc.tile_pool(name="junk", bufs=2) as jpool,
            tc.tile_pool(name="small", bufs=1) as spool,
            tc.tile_pool(name="gat", bufs=4) as gpool,
        ):
            wha = cpool.tile([128, K_CODES], r32, tag="wha")
            nc.sync.dma_start(wha[:], whT[0:128, :])
            whb = cpool.tile([128, K_CODES], r32, tag="whb")
            nc.sync.dma_start(whb[:], whT[128:256, :])
            wla = cpool.tile([128, K_CODES], r32, tag="wla")
            nc.gpsimd.dma_start(wla[:], wlT[0:128, :])
            wlb = cpool.tile([128, K_CODES], r32, tag="wlb")
            nc.gpsimd.dma_start(wlb[:], wlT[128:256, :])
            nhl_t = cpool.tile([2, K_CODES], r32, tag="nhl")
            nc.gpsimd.dma_start(nhl_t[:], nhl[:])
            ones_t = cpool.tile([2, 128], f32, tag="ones")
            nc.vector.memset(ones_t[:], 1.0)
            ones = ones_t[:].bitcast(r32)
            zeros = cpool.tile([128, K_CODES], f32, tag="zeros")
            nc.vector.memset(zeros[:], 0.0)

            idxf = spool.tile([128, TILES], f32, tag="idxf")
            idxi = spool.tile([128, TILES], mybir.dt.int32, tag="idxi")
            vmaxb = spool.tile([128, TILES], f32, tag="vmaxb")

            zsrc = zsp.rearrange("(c p) t -> p c t", p=128)
            for tt in range(TILES):
                zt4 = zpool.tile([128, 512], r32, tag="zt4")
                nc.sync.dma_start(
                    zt4[:].rearrange("p (c j) -> p c j", c=4),
                    zsrc[:, :, tt * 128 : (tt + 1) * 128],
                )
                zh0 = zt4[:, 0:128]
                zh1 = zt4[:, 128:256]
                zl0 = zt4[:, 256:384]
                zl1 = zt4[:, 384:512]

                q = qpool.tile([128, K_CODES], f32, tag="q")
                L, R = slice(0, 512), slice(512, 1024)
                mm = nc.tensor.matmul
                # grouped by stationary operand (5 weight loads per tile)
                mm(q[:, L], zh0, wha[:, L], start=True, stop=False)
                mm(q[:, R], zh0, wha[:, R], start=True, stop=False)
                mm(q[:, L], zh0, wla[:, L], start=False, stop=False)
                mm(q[:, R], zh0, wla[:, R], start=False, stop=False)
                mm(q[:, L], zh1, whb[:, L], start=False, stop=False)
                mm(q[:, R], zh1, whb[:, R], start=False, stop=False)
                mm(q[:, L], zh1, wlb[:, L], start=False, stop=False)
                mm(q[:, R], zh1, wlb[:, R], start=False, stop=False)
                mm(q[:, L], zl0, wha[:, L], start=False, stop=False)
                mm(q[:, R], zl0, wha[:, R], start=False, stop=False)
                mm(q[:, L], zl1, whb[:, L], start=False, stop=False)
                mm(q[:, R], zl1, whb[:, R], start=False, stop=False)
                mm(q[:, L], ones[0:2, :], nhl_t[0:2, L], start=False, stop=True)
                mm(q[:, R], ones[0:2, :], nhl_t[0:2, R], start=False, stop=True)

                pm = pmpool.tile([128, K_CODES], f32, tag="pm")
                nc.vector.tensor_tensor_scan(
                    out=pm[:],
                    data0=q[:],
                    data1=zeros[:],
                    initial=-3.0e38,
                    op0=mybir.AluOpType.max,
                    op1=mybir.AluOpType.bypass,
                )
                # idx = sum_k Sign(tmax - pm[k]) on the scalar engine
                junk = jpool.tile([128, K_CODES], f32, tag="junk")
                nc.scalar.activation(
                    junk[:],
                    pm[:],
                    mybir.ActivationFunctionType.Sign,
                    bias=pm[:, K_CODES - 1 : K_CODES],
                    scale=-1.0,
                    accum_out=idxf[:, tt : tt + 1],
                )
                # row max for the loss
                nc.scalar.copy(vmaxb[:, tt : tt + 1], pm[:, K_CODES - 1 : K_CODES])

                # f32 -> int32 cast and z_q gather, pipelined per tile
                # (cast on gpsimd: keeps DVE free for the scans and chains
                # naturally into the gpsimd-issued indirect DMA)
                nc.gpsimd.tensor_copy(idxi[:, tt : tt + 1], idxf[:, tt : tt + 1])
                g = gpool.tile([128, D_DIM], f32, tag="g")
                nc.gpsimd.indirect_dma_start(
                    out=g[:],
                    out_offset=None,
                    in_=wsrc[:],
                    in_offset=bass.IndirectOffsetOnAxis(
                        ap=idxi[:, tt : tt + 1], axis=0
                    ),
                )
                nc.sync.dma_start(zq[tt * 128 : (tt + 1) * 128, :], g[:])

            nc.sync.dma_start(idxo[:], idxi[:])
            nc.sync.dma_start(vmaxo[:], vmaxb[:])

    return nc


def _trunc10(x):
    return (x.view(np.int32) & np.int32(~((1 << 13) - 1))).view(np.float32)


def _prep_inputs(z, weight):
    zT = np.ascontiguousarray(z.T)                         # [256, 65536]
    zhT_full = _trunc10(zT)
    zlT_full = zT - zhT_full
    zsp_full = np.concatenate([zhT_full, zlT_full], axis=0)  # [512, 65536]
    w2T = np.ascontiguousarray((2.0 * weight).T)           # [256, 1024]
    whT = _trunc10(w2T)
    wlT = np.ascontiguousarray(w2T - whT)
    negwsq = -(weight.astype(np.float64) ** 2).sum(axis=1).astype(np.float32)[None, :]
    nwh = _trunc10(negwsq)
    nhl = np.ascontiguousarray(np.concatenate([nwh, negwsq - nwh], axis=0))
    in_maps = []
    for c in range(N_CORES):
        sl = slice(c * TOK_PER_CORE, (c + 1) * TOK_PER_CORE)
        in_maps.append(
            {
                "zsp": np.ascontiguousarray(zsp_full[:, sl]),
                "whT": whT,
                "wlT": wlT,
                "nhl": nhl,
                "wsrc": weight,
            }
        )
    return in_maps


def kernel(z: np.ndarray, weight: np.ndarray):
    z = np.ascontiguousarray(z, dtype=np.float32)
    weight = np.ascontiguousarray(weight, dtype=np.float32)
    assert z.shape == (N_TOK, D_DIM) and weight.shape == (K_CODES, D_DIM)

    if "nc" not in _cache:
        _cache["nc"] = _build_bass()
    nc = _cache["nc"]

    from concourse.bass_utils import run_bass_kernel_spmd

    in_maps = _prep_inputs(z, weight)
    res = run_bass_kernel_spmd(nc, in_maps, core_ids=list(range(N_CORES)))
    return _assemble(z, weight, [res.results[c] for c in range(N_CORES)])


def _assemble(z, weight, results):
    z_q = np.concatenate([r["zq"] for r in results], axis=0)
    idx = np.concatenate(
        [r["idxo"].T.reshape(-1) for r in results], axis=0
    ).astype(np.int32)
    # loss = BETA * mean(d_min); d_min[n] = ||z_n||^2 - max_k q[n,k]
    sum_vmax = sum(float(r["vmaxo"].sum(dtype=np.float64)) for r in results)
    sum_zsq = float((z.astype(np.float64) ** 2).sum())
    loss = np.float32(BETA * (sum_zsq - sum_vmax) / (N_TOK * D_DIM))
    # straight-through estimator, matching reference arithmetic exactly
    z_q_st = z + (z_q - z)
    return (loss, z_q_st, idx)
